# revision 3
# baseline (speedup 1.0000x reference)
"""BitLinear (fake-quant straight-through) Trainium2 kernel, v3.

Math (per the reference nn module):
  dqx = round(x * s_x) / s_x         s_x = 127 / clip(rowabsmax(x), 1e-5)   (per token row)
  dqw = clip(round(w * s_w), -1, 1) / s_w    s_w = 1 / clip(mean(|w|), 1e-5)  (per tensor)
  out = dqx @ dqw.T + bias

Key facts this kernel exploits:
  * round(x*s_x) is an integer in [-127, 127] and clip(round(w*s_w)) is in
    {-1, 0, 1}; both are EXACT in bf16, and the matmul accumulates in fp32
    PSUM where all partial sums (<= 2^17) are exact integers.  The heavy
    matmul runs at bf16 PE rate with zero quantization-path error; the
    per-token / per-tensor scales are applied to the (exact) integer matmul
    result at PSUM-evacuation time.
  * round-half-even == fp32 RNE, so `round(v)` is computed exactly as
    `(v + 1.5*2^23) - 1.5*2^23` with two fp32 ALU stages (no Round op needed).

Sharding: data parallel over the batch dim; core i computes batch element i
with the full weight.  No collectives; the host scatters x and gathers out.

Pipeline (one 128-token tile per step, 32 steps/core).  The input stage for
tile t+2 is emitted in the same loop iteration as the matmul/evacuation for
tile t, so each engine's in-order queue keeps the input side ~2 tiles (7 us)
ahead of the PE and the output side never blocks it.  x loads are dispatched
4 tiles ahead of their consumer for the same reason.

Engine assignment (steady-state per-tile cost vs the 3.46 us matmul slot):
  sync   : x input DMA dispatch + xbar transpose        (~2.0 us)
  vector : absmax reduce, scales, round via magic const (~2.4 us)
  scalar : -MAGIC+bf16 convert, fs, PSUM evacuation     (~3.1 us)
  tensor : 16 matmuls [128k x 128s x 512n]              (3.46 us)
  gpsimd : bias add (Pool) + output store (SWDGE)       (~2.8 us)

Host-side static prep (weights are per-call constants): s_w (must match the
reference's fp32 mean to ~1 ulp), the ternary weight pre-transposed into the
matmul rhs layout (bf16, exact), the bias broadcast row, and the scalar
k1 = 1/(127*s_w) that turns the per-token absmax into the output scale.
Output is stored bf16 (adds ~2^-9 relative error vs the 2e-2 gate) and
upcast on the host.
"""

import numpy as np

from concourse import bacc, bass, mybir, tile
from concourse.bass_utils import run_bass_kernel_spmd

F32 = mybir.dt.float32
BF16 = mybir.dt.bfloat16
ALU = mybir.AluOpType
ACTF = mybir.ActivationFunctionType

MAGIC = 12582912.0  # 1.5 * 2**23: fp32 RNE round-to-integer constant
EPS = 1e-05

B, S, K, N = 8, 4096, 1024, 1024
N_CORES = 8
LOOK = 4  # x-DMA dispatch lookahead (tiles)
PIPE = 2  # input-stage lookahead vs matmul stage (tiles)


def build(s_tokens=S, k=K, n=N):
    """Build the single-core SPMD program: x[s_tokens,k] @ w[n,k]^T quantized."""
    nc = bacc.Bacc("TRN2", target_bir_lowering=False, debug=False)

    KT = k // 128          # contraction tiles
    NT = n // 128          # weight row tiles
    NH = n // 512          # psum-bank halves of the output feature dim
    NS = s_tokens // 128   # token tiles

    x_d = nc.dram_tensor("x", [s_tokens, k], F32, kind="ExternalInput").ap()
    # qwt: host-ternarized weight, pre-transposed to the rhs layout
    # [kpart, kt, nt, n128]: element (p, kt, nt, j) = qw[n=nt*128+j, k=kt*128+p]
    qwt_d = nc.dram_tensor("qwt", [128, KT * n], BF16, kind="ExternalInput").ap()
    # bias broadcast to all 128 partitions (bf16)
    biasb_d = nc.dram_tensor("biasb", [128, n], BF16, kind="ExternalInput").ap()
    consts_d = nc.dram_tensor("consts", [128, 2], F32, kind="ExternalInput").ap()
    out_d = nc.dram_tensor("out", [s_tokens, n], BF16, kind="ExternalOutput").ap()

    x_t = x_d.rearrange("(t p) k -> t p k", p=128)
    out_t = out_d.rearrange("(t p) n -> t p n", p=128)

    with tile.TileContext(nc) as tc:
        with (
            tc.tile_pool(name="static", bufs=1) as static,
            tc.tile_pool(name="xpool", bufs=6) as xpool,
            tc.tile_pool(name="qpool", bufs=3) as qpool,
            tc.tile_pool(name="qtpool", bufs=4) as qtpool,
            tc.tile_pool(name="opool", bufs=3) as opool,
            tc.tile_pool(name="vpool", bufs=6) as vpool,
            tc.tile_pool(name="psum", bufs=4, space="PSUM") as psum_pool,
        ):
            consts = static.tile([128, 2], F32)
            nc.scalar.dma_start(consts[:], consts_d[:])
            k1 = consts[:, 0:1]  # (1/s_w) / 127  (output scale factor)

            biasb = static.tile([128, n], BF16)
            nc.scalar.dma_start(biasb[:], biasb_d[:])

            # weight rhs [128, kt, nt, 128], loaded in 4 chunks so the kt=0
            # half is ready ~1.5us in
            qwt = static.tile([128, KT, NT, 128], BF16)
            qwt_f = qwt[:].rearrange("p kt nt j -> p (kt nt j)")
            CH = KT * n // 4
            for c in range(4):
                nc.scalar.dma_start(
                    qwt_f[:, c * CH:(c + 1) * CH], qwt_d[:, c * CH:(c + 1) * CH]
                )

            xs = {}

            def load_x(t):
                xs[t] = xpool.tile([128, k], F32, name="x_s")
                nc.sync.dma_start(xs[t][:], x_t[t])

            fss = {}
            qxTs = {}

            def input_stage(t):
                x_s = xs[t]
                c = vpool.tile([128, 1], F32, name="c")
                nc.vector.tensor_reduce(
                    c[:], x_s[:], mybir.AxisListType.X, ALU.max,
                    apply_absolute_value=True,
                )
                cc = vpool.tile([128, 1], F32, name="cc")
                nc.vector.tensor_scalar_max(cc[:], c[:], EPS)
                rc = vpool.tile([128, 1], F32, name="rc")
                nc.vector.reciprocal(rc[:], cc[:])
                ss = vpool.tile([128, 1], F32, name="ss")
                nc.vector.tensor_scalar_mul(ss[:], rc[:], 127.0)
                # output scale fs = cc * k1, produced on the scalar engine
                fs = fss[t] = vpool.tile([128, 1], F32, name="fs")
                nc.scalar.activation(fs[:], cc[:], ACTF.Copy, scale=k1)

                # round(x*s_x) via magic constant, in place on x_s (DVE);
                # the -MAGIC correction + bf16 convert runs on the scalar
                # engine (exact: result is a small integer)
                nc.vector.tensor_scalar(
                    x_s[:], x_s[:], ss[:], MAGIC, ALU.mult, ALU.add,
                )
                qx = qpool.tile([128, k], BF16, name="qx")
                nc.scalar.activation(qx[:], x_s[:], ACTF.Copy, bias=-MAGIC)

                # xbar transpose: [128s, 1024k] -> [128k, KT, 128s]
                qxT = qxTs[t] = qtpool.tile([128, KT, 128], BF16, name="qxT")
                nc.sync.dma_start_transpose(qxT[:], qx[:])

            def mm_stage(t):
                qxT = qxTs.pop(t)
                fs = fss.pop(t)
                outs = opool.tile([128, n], BF16, name="outs")
                ps_list = [
                    psum_pool.tile([128, 512], F32, name=f"ps{h}", tag=f"ps{h}")
                    for h in range(NH)
                ]
                for kt in range(KT):
                    for h in range(NH):
                        nc.tensor.matmul(
                            ps_list[h][:],
                            qxT[:, kt, :],
                            qwt[:, kt, 4 * h:4 * h + 4, :],
                            start=(kt == 0),
                            stop=(kt == KT - 1),
                        )
                for h in range(NH):
                    nc.scalar.activation(
                        outs[:, h * 512:(h + 1) * 512], ps_list[h][:],
                        ACTF.Copy, scale=fs[:],
                    )
                nc.gpsimd.tensor_add(outs[:], outs[:], biasb[:])
                nc.gpsimd.dma_start(out_t[t], outs[:])

            for t in range(LOOK):
                load_x(t)
            for t in range(PIPE):
                input_stage(t)
            for t in range(NS):
                if t + LOOK < NS:
                    load_x(t + LOOK)
                if t + PIPE < NS:
                    input_stage(t + PIPE)
                mm_stage(t)

    nc.compile()
    return nc


def host_prep(weight, bias):
    """Host-side static weight prep: s_w, ternary pre-transposed weight, bias.

    s_w must match the reference's fp32 mean reduction (jnp.mean on f32) to
    ~1 ulp, so it is computed with the same jax op on CPU when available.
    The ternarization then reproduces the reference exactly: fp32 multiply
    by s_w, fp32 round-half-even, clip to [-1, 1].
    """
    import ml_dtypes

    w = np.ascontiguousarray(weight, dtype=np.float32)
    try:
        import jax
        import jax.numpy as jnp

        with jax.default_device(jax.devices("cpu")[0]):
            mean_abs = np.float32(
                jax.device_get(jnp.mean(jnp.abs(jnp.asarray(w, dtype=jnp.float32))))
            )
    except Exception:
        mean_abs = np.float32(np.mean(np.abs(w), dtype=np.float32))
    mean_c = np.maximum(mean_abs, np.float32(EPS))
    sw = np.float32(1.0) / mean_c          # s_w, the weight quant scale
    k1 = (np.float32(1.0) / sw) / np.float32(127.0)  # output scale = cc * k1

    qw = np.clip(np.round(w * sw), -1.0, 1.0).astype(np.float32)  # [n, k] ternary
    # rhs layout [128 kpart, KT, NT, 128n]
    KT, NT = K // 128, N // 128
    qwt = (
        qw.T.reshape(KT, 128, NT, 128)       # [kt, kpart, nt, j]
        .transpose(1, 0, 2, 3)               # [kpart, kt, nt, j]
        .reshape(128, KT * N)
        .astype(ml_dtypes.bfloat16)
    )

    b = np.asarray(bias, dtype=np.float32).astype(ml_dtypes.bfloat16)
    biasb = np.tile(b[None, :], (128, 1))

    consts = np.zeros((128, 2), np.float32)
    consts[:, 0] = k1
    return qwt.copy(), biasb.copy(), consts


_NC_CACHE = {}


def _get_nc():
    if "nc" not in _NC_CACHE:
        _NC_CACHE["nc"] = build()
    return _NC_CACHE["nc"]


def make_in_maps(x, weight, bias):
    x = np.ascontiguousarray(x, dtype=np.float32)
    qwt, biasb, consts = host_prep(weight, bias)
    return [
        {"x": x[i], "qwt": qwt, "biasb": biasb, "consts": consts}
        for i in range(N_CORES)
    ]


def kernel(x, weight, bias, **kwargs):
    nc = _get_nc()
    in_maps = make_in_maps(x, weight, bias)
    last_err = None
    for _attempt in range(3):
        try:
            res = run_bass_kernel_spmd(nc, in_maps, list(range(N_CORES)))
            return np.stack(
                [res.results[i]["out"].astype(np.float32) for i in range(N_CORES)],
                axis=0,
            )
        except Exception as e:  # transient NRT device errors: retry
            last_err = e
    raise last_err
